# revision 15
# baseline (speedup 1.0000x reference)
"""Trainium2 Bass kernel for nn_MultiHeadAttention_46188078301212.

Module semantics (replicated from the PyTorch module's quirky reshape):
  P_q = q @ Wq.T + bq  (same for k, v), each [B, 2048, 512]
  Head h takes projection rows [256h, 256h+256) viewed as [2048, 64]
  (row-major), runs standard softmax attention, heads are concatenated
  along hidden (out col block 64h..64h+64) and merged with Wm.T + bm.

Sharding: 8 cores = (batch b in {0,1}) x (head-pair j4 in {0..3}).
Core c handles b = c//4 and global heads {2*j4, 2*j4+1}, which only need
projection rows [512*j4, 512*j4+512) of q/k/v for that batch.  Each core
emits, per head, an UNNORMALIZED partial [2048, 512] (the head's
contribution to the merge matmul, numerator only) plus the softmax
denominator row; the host divides by the denominators, sums the 16
head partials per batch, and adds bm.

On-core layout notes:
  - Head-local index is permuted: q' = c*256 + s (c = hidden chunk e//64,
    s = local row).  Softmax is permutation-invariant; the final DMA
    un-permutes rows back to the true order.
  - Scores are computed transposed (S^T: keys on partitions, queries on
    free axis) so that P@V can contract over keys on the partition axis.
  - Attention runs in two q'-half phases; kappa' blocks [0,1024) and
    [1024,2048) live on partitions 0:64 / 64:128 and are contracted with
    K=64 quadrant matmuls (PE tile (0,0) / (64,0)), so no zero-padding
    of the Q tiles is needed.
  - Softmax denominators come from an all-ones column appended to V and
    ride along to the host in the bf16 output.
  - All matmuls are bf16 (psum accumulation f32); probs are bf16.
"""

import numpy as np

HIDDEN = 512
DHEAD = 64
B = 2
S = 2048
NCORES = 8
NWARM = 8

_CACHE = {}

# Filled in by the last kernel() call when BASS_TRACE is set.
LAST_EXEC_NS = None
LAST_RESULTS = None


def _build_nc():
    if "nc" in _CACHE:
        return _CACHE["nc"]

    import contextlib

    import concourse.mybir as mybir
    import concourse.tile as tile
    from concourse import bacc

    f32 = mybir.dt.float32
    bf16 = mybir.dt.bfloat16
    Exp = mybir.ActivationFunctionType.Exp
    Copy = mybir.ActivationFunctionType.Copy
    Identity = mybir.ActivationFunctionType.Identity

    nc = bacc.Bacc("TRN2", target_bir_lowering=False)

    # ---- DRAM I/O ----
    # big inputs arrive pre-arranged as [128, t, d] so every DMA line is
    # contiguous per partition
    d_x = {}
    for nm in ("q", "k", "v"):
        d_x[nm] = nc.dram_tensor(f"x{nm}T", [128, 4, 512], bf16, kind="ExternalInput")
    # wq is column-chunk-major [p, cq, D, d] so each 128-col chunk is its
    # own contiguous DMA and proj_q(cq) can start as soon as it lands
    d_wq = nc.dram_tensor("wqT", [128, 8, 4, 128], bf16, kind="ExternalInput")
    d_wk = nc.dram_tensor("wkT", [128, 4, 512], bf16, kind="ExternalInput")
    d_wv = nc.dram_tensor("wvT", [128, 4, 512], bf16, kind="ExternalInput")
    d_wm = [
        nc.dram_tensor("wm0", [128, 512], bf16, kind="ExternalInput"),
        nc.dram_tensor("wm1", [128, 512], bf16, kind="ExternalInput"),
    ]
    d_bq = nc.dram_tensor("bq8", [128, 8], f32, kind="ExternalInput")
    d_bk = nc.dram_tensor("bk4", [128, 4], f32, kind="ExternalInput")
    # Per-head unnormalized merge partials; rows permuted exactly like the
    # old combined output (row = 1024*(ii%2) + 8*m + (4H + ii//2)).
    d_out = nc.dram_tensor("outp", [2, 2048, 512], bf16, kind="ExternalOutput")
    d_out_r = d_out.rearrange("t (b p a) e -> p t b a e", b=2, p=128)
    # Per-class prob masses per phase pi=2t+H: dens[pi, c, j] = sum of probs
    # over kappa in residue class c (kappa%8 == c), for in-stream col j.
    # Host: denominator = sum_c, V-bias term = sum_c m_c * (bv_c @ Wm_h^T).
    d_den = nc.dram_tensor("dens", [4, 8, 1024], bf16, kind="ExternalOutput")

    with tile.TileContext(nc) as tc:
        ctx = contextlib.ExitStack()
        with ctx:
            big = ctx.enter_context(tc.tile_pool(name="big", bufs=8))
            sp = ctx.enter_context(tc.tile_pool(name="sp", bufs=2, space="PSUM"))
            op = ctx.enter_context(tc.tile_pool(name="op", bufs=2, space="PSUM"))
            pp = ctx.enter_context(tc.tile_pool(name="pp", bufs=2, space="PSUM"))
            ptp = ctx.enter_context(tc.tile_pool(name="ptp", bufs=12))
            small = ctx.enter_context(tc.tile_pool(name="small", bufs=1))
            otp = ctx.enter_context(tc.tile_pool(name="otp", bufs=3))

            # ---- tiny constants ----
            # warmup operand row FIRST on the DVE queue: the K=1 warmup
            # matmuls depend only on this ~60ns memset, so the PE can start
            # within ~0.3us of the DMA-ring init instead of waiting for a
            # big [128,512] memset queued behind other DVE work.
            wrow = small.tile([1, 640], bf16, tag="wrow")
            nc.vector.memset(wrow, 0.0)
            ones_tiny = small.tile([1, 1], f32, tag="ones_tiny")
            nc.vector.memset(ones_tiny, 1.0)
            warm = small.tile([1, 1], f32, tag="warm")
            nc.scalar.activation(warm, ones_tiny, Exp, scale=1.0)

            # ---- input DMAs, spread over engine queues (NOT scalar: the
            # scalar engine runs the exp stream and must stay exp-only) so
            # the critical projections (q then k) can start early ----
            xin = {}
            # Each dma_start costs its engine ~0.2-1.3us of descriptor-gen
            # time, so keep doorbell counts low and earliest-needed first.
            # sync queue: xq first (gates the upfront projections), the
            # small bias tensors, then xk (needed by proj_k(0) at ~7us)
            xt = big.tile([128, 4, 512], bf16, tag="big", name="xq")
            nc.sync.dma_start(out=xt, in_=d_x["q"][:, :, :])
            bq8 = small.tile([128, 8], f32, tag="bq8")
            nc.sync.dma_start(out=bq8, in_=d_bq[:, :])
            bk4 = small.tile([128, 4], f32, tag="bk4")
            nc.sync.dma_start(out=bk4, in_=d_bk[:, :])
            xkt = big.tile([128, 4, 512], bf16, tag="big", name="xk")
            nc.sync.dma_start(out=xkt, in_=d_x["k"][:, :, :])
            xin["q"] = xt
            xin["k"] = xkt

            # scalar queue (exp stream only begins once the upfront
            # projections finish): wq first half in two 2-chunk DMAs, then
            # wk; wm last (not needed before ~25us)
            wqt = big.tile([128, 8, 4, 128], bf16, tag="big", name="wq")
            nc.scalar.dma_start(out=wqt[:, 0:2, :, :], in_=d_wq[:, 0:2, :, :])
            nc.scalar.dma_start(out=wqt[:, 2:4, :, :], in_=d_wq[:, 2:4, :, :])
            wkt = big.tile([128, 4, 512], bf16, tag="big", name="wk")
            nc.scalar.dma_start(out=wkt, in_=d_wk[:, :, :])
            wm_sb = []
            for t in range(2):
                w = small.tile([128, 512], bf16, tag=f"wm{t}", name=f"wm{t}")
                nc.scalar.dma_start(out=w, in_=d_wm[t][:, :])
                wm_sb.append(w)

            # gpsimd queue: the v-side and the deferred wq second half,
            # none needed before ~15us
            wvt = big.tile([128, 4, 512], bf16, tag="big", name="wv")
            nc.gpsimd.dma_start(out=wvt, in_=d_wv[:, :, :])
            xvt = big.tile([128, 4, 512], bf16, tag="big", name="xv")
            nc.gpsimd.dma_start(out=xvt, in_=d_x["v"][:, :, :])
            nc.gpsimd.dma_start(out=wqt[:, 4:8, :, :], in_=d_wq[:, 4:8, :, :])
            xin["v"] = xvt
            win = {"q": wqt, "k": wkt, "v": wvt}

            # ---- per-head working tensors ----
            # QLO rows 0:64 = Q^T (chunk c at cols 256c); rows 64:128 unused
            # QHI rows 64:128 = same Q^T duplicate; rows 0:64 unused
            # KT[t]: [128, 1024]  rows 0:64 = K^T kappa' [0,1024) (chunk c at
            #                     cols 256c); rows 64:128 = kappa' [1024,2048)
            # V[t]:  [128, 16, 72] bf16; [:, jj, 0:64] = V chunk jj,
            #        col 64+jj//2 = 1 (class indicator), rest of 64:72 = 0
            OTb = [
                small.tile([128, 1024], bf16, tag=f"OTb{i}", name=f"OTb{i}")
                for i in range(2)
            ]
            for i in range(2):
                # rows 64:128 pair with zero rows of wm in the merge matmul;
                # zero them once so stale NaN patterns can't poison 0*x
                # (rows 64:72 are re-written with class masses by head_copy)
                nc.gpsimd.memset(OTb[i][64:128, :], 0.0)
            # QD: ONE tensor holding Q^T duplicated on both partition bands
            # (rows 0:64 == rows 64:128).  Keeping both bands in a single
            # tensor lets the kappa-lo/kappa-hi score matmuls of a pair
            # stream the SAME free-axis columns from complementary bands,
            # which is the pattern the PE row-tiling concurrency needs.
            QD = small.tile([128, 2, 2048], bf16, tag="QD", name="QD")
            KT2 = small.tile([128, 2, 1024], bf16, tag="KT2", name="KT2")
            # V tiles padded to 128 weight columns: a 128-col bf16 weight
            # load gets FWL (fast weight load), halving the LDW cost the
            # PV matmuls pay (their full-row weight loads cannot overlap
            # any in-flight matmul).  Cols 72:128 are zero -> O rows
            # 72:128 accumulate zeros and are never read.
            Vt = [
                small.tile([128, 16, 128], bf16, tag=f"V{t}", name=f"V{t}")
                for t in range(2)
            ]
            for t in range(2):
                nc.vector.memset(Vt[t][:, :, 64:128], 0.0)
                for c in range(8):
                    nc.vector.memset(
                        Vt[t][:, 2 * c : 2 * c + 2, 64 + c : 65 + c], 1.0
                    )

            # ---- projection emitters (closures; some upfront, rest
            # spread into the attention stream to overlap with it) ----
            def proj_q(cq, use_act=False):
                xt, wt = xin["q"], win["q"]
                ps = pp.tile([128, 512], f32, tag="pp", name="psq")
                for D in range(4):
                    nc.tensor.matmul(
                        ps,
                        wt[:, cq, D, :],
                        xt[:, D, :],
                        start=(D == 0),
                        stop=(D == 3),
                    )
                psr = ps.rearrange("p (t s) -> p t s", t=2)
                nc.vector.tensor_scalar_add(
                    QD[0:64, :, 256 * cq : 256 * cq + 256],
                    psr[0:64, :, :],
                    bq8[0:64, cq : cq + 1],
                )
                if use_act:
                    # startup only: route the second half through ScalarE so
                    # the serial DVE bias chain isn't the critical path
                    nc.scalar.activation(
                        QD[64:128, :, 256 * cq : 256 * cq + 256],
                        psr[64:128, :, :],
                        Identity,
                        bias=bq8[64:128, cq : cq + 1],
                    )
                else:
                    nc.vector.tensor_scalar_add(
                        QD[64:128, :, 256 * cq : 256 * cq + 256],
                        psr[64:128, :, :],
                        bq8[64:128, cq : cq + 1],
                    )

            def proj_k(cp):
                xt, wt = xin["k"], win["k"]
                ps = pp.tile([128, 512], f32, tag="pp", name="psk")
                for D in range(4):
                    nc.tensor.matmul(
                        ps,
                        wt[:, D, 128 * cp : 128 * cp + 128],
                        xt[:, D, :],
                        start=(D == 0),
                        stop=(D == 3),
                    )
                nc.vector.tensor_scalar_add(
                    KT2[:, :, 256 * cp : 256 * cp + 256],
                    ps.rearrange("p (t s) -> p t s", t=2),
                    bk4[:, cp : cp + 1],
                )

            def proj_v(St):
                # bv is folded into bm on the host: through attention the
                # V-bias contributes den[q] * (Wm_h @ bv_h), which after
                # normalization is the constant vector Wm @ bv.
                xt, wt = xin["v"], win["v"]
                ps = pp.tile([128, 512], f32, tag="pp", name="psv")
                for D in range(4):
                    nc.tensor.matmul(
                        ps,
                        xt[:, D, 128 * St : 128 * St + 128],
                        wt[:, D, :],
                        start=(D == 0),
                        stop=(D == 3),
                    )
                t, half = St // 2, St % 2
                for c in range(8):
                    nc.vector.tensor_copy(
                        Vt[t][:, 2 * c + half, 0:64], ps[:, 64 * c : 64 * c + 64]
                    )

            # PE warm-up on a K=1 zero row: back-to-back matmuls so the HAM
            # un-throttles; bridges the input-DMA window.
            wups = pp.tile([128, 512], f32, tag="pp", name="wups")
            for i in range(NWARM):
                nc.tensor.matmul(
                    wups,
                    wrow[:, 0:128],
                    wrow[:, 128:640],
                    start=(i == 0),
                    stop=(i == NWARM - 1),
                )

            from collections import deque

            proj_todo = deque()
            # upfront: everything phase (t=0, H=0) needs
            for cq in range(4):
                proj_q(cq, use_act=True)
            proj_k(0)
            # rest spread into the stream (ordered to match DMA arrivals:
            # Q chunks 4..7 last, since the second wq half lands latest)
            proj_todo.append(lambda: proj_k(1))
            proj_todo.append(lambda: proj_k(2))
            proj_todo.append(lambda: proj_k(3))
            proj_todo.append(lambda: proj_v(0))
            proj_todo.append(lambda: proj_v(1))
            proj_todo.append(lambda: proj_v(2))
            proj_todo.append(lambda: proj_v(3))
            for cq in range(4, 8):
                proj_todo.append(lambda cq=cq: proj_q(cq))

            # ---- attention + merge, 4 phases (head, q'-half), with PV
            # lagged LAG steps behind the S/exp stream so PE never blocks
            # on freshly produced probs ----
            phases = [(t, H) for t in range(2) for H in range(2)]
            O_tiles = {}
            LAG = 6
            pend = deque()

            merge_q = deque()

            def do_merge(pi):
                # Copy O^T (rows 0:65, incl. denominator row 64) to SBUF
                # bf16, DMA the den row, then 8 merge matmuls whose psum is
                # cast to bf16 and DMA'd out unnormalized.  All queued to be
                # spread across subsequent stream steps.
                t, H = phases[pi]
                Oa, Ob = O_tiles[pi]
                OT = OTb[pi % 2]

                last = pi == len(phases) - 1

                def head_copy(Oa=Oa, Ob=Ob, OT=OT, pi=pi, last=last):
                    nc.vector.tensor_copy(OT[0:72, 0:512], Oa[0:72, :])
                    if last:
                        # exp stream is done: ACT takes the second copy
                        nc.scalar.activation(
                            OT[0:72, 512:1024], Ob[0:72, :], Copy, scale=1.0
                        )
                    else:
                        nc.vector.tensor_copy(OT[0:72, 512:1024], Ob[0:72, :])
                    nc.gpsimd.dma_start(
                        out=d_den[pi, :, :], in_=OT[64:72, :]
                    )

                merge_q.append(head_copy)

                def merge_tile(ii, t=t, H=H, OT=OT, last=last):
                    # in the final drain the O psum banks are free after
                    # head_copy, so draw from the op pool too: 4 in-flight
                    # merge tiles instead of 2, so PE doesn't stall on casts
                    pool = op if (last and ii % 2 == 1) else pp
                    mp = pool.tile(
                        [128, 512], f32, tag="op" if pool is op else "pp", name="mp"
                    )
                    nc.tensor.matmul(mp, OT[:, 128 * ii : 128 * ii + 128], wm_sb[t])
                    return mp

                def cast_pair(iip, mpa, mpb, ob, last=last):
                    # psum -> sbuf bf16 (GPSIMD cannot read PSUM).  In the
                    # final drain the exp stream is done, so ACT helps out
                    # and each tile is DMA'd as soon as it is cast.
                    if last:
                        nc.vector.tensor_copy(ob[:, 0, :], mpa)
                        enga = (nc.sync, nc.gpsimd, nc.scalar, nc.sync)[iip]
                        enga.dma_start(
                            out=d_out_r[:, t, 0, 4 * H + iip, :], in_=ob[:, 0, :]
                        )
                        nc.scalar.activation(ob[:, 1, :], mpb, Copy, scale=1.0)
                        engb = (nc.gpsimd, nc.scalar, nc.sync, nc.gpsimd)[iip]
                        engb.dma_start(
                            out=d_out_r[:, t, 1, 4 * H + iip, :], in_=ob[:, 1, :]
                        )
                    else:
                        nc.vector.tensor_copy(ob[:, 0, :], mpa)
                        nc.vector.tensor_copy(ob[:, 1, :], mpb)
                        eng = nc.sync if iip % 2 == 0 else nc.gpsimd
                        eng.dma_start(
                            out=d_out_r[:, t, :, 4 * H + iip, :], in_=ob
                        )

                state = {}

                def mk_mm(ii):
                    def f():
                        state[ii] = merge_tile(ii)

                    return f

                def mk_cast(iip):
                    def f():
                        ob = otp.tile([128, 2, 512], bf16, tag="ob", name="ob")
                        cast_pair(iip, state.pop(2 * iip), state.pop(2 * iip + 1), ob)

                    return f

                for iip in range(4):
                    merge_q.append(mk_mm(2 * iip))
                    merge_q.append(mk_mm(2 * iip + 1))
                    merge_q.append(mk_cast(iip))

            def issue_pv(pi, k, pt):
                t, H = phases[pi]
                jp, half = k // 2, k % 2
                jj = jp + 8 * half
                for n in range(2):
                    nc.tensor.matmul(
                        O_tiles[pi][n][0:128, :],
                        Vt[t][:, jj, :],
                        pt[:, 512 * n : 512 * n + 512],
                        start=(k == 0),
                        stop=(k == 15),
                    )
                if k == 15:
                    do_merge(pi)

            for pi, (t, H) in enumerate(phases):
                O_tiles[pi] = (
                    op.tile([128, 512], f32, tag="op", name="oaugA"),
                    op.tile([128, 512], f32, tag="op", name="oaugB"),
                )
                for jp in range(8):
                    for _ in range(2):
                        if proj_todo:
                            proj_todo.popleft()()
                    # kappa' chunk 128*jp (lo half, PE row-tile (0,0)) and
                    # 1024+128*jp (hi half, row-tile (64,0)).  The two MMs
                    # of each n are issued back-to-back streaming the SAME
                    # QD columns from complementary partition bands so the
                    # PE can run them concurrently in the two row groups.
                    sTa = sp.tile([128, 1024], f32, tag="sp", name="sTa")
                    sTb = sp.tile([128, 1024], f32, tag="sp", name="sTb")
                    for n in range(2):
                        qap = QD[
                            :, t, 1024 * H + 512 * n : 1024 * H + 512 * n + 512
                        ]
                        nc.tensor.matmul(
                            sTa[:, 512 * n : 512 * n + 512],
                            KT2[0:64, t, 128 * jp : 128 * jp + 128],
                            qap[0:64],
                        )
                        nc.tensor.matmul(
                            sTb[:, 512 * n : 512 * n + 512],
                            KT2[64:128, t, 128 * jp : 128 * jp + 128],
                            qap[64:128],
                        )
                    for half, sT in ((0, sTa), (1, sTb)):
                        k = 2 * jp + half
                        pt = ptp.tile([128, 1024], bf16, tag="pt")
                        nc.scalar.activation(pt, sT, Exp, scale=0.125)
                        pend.append((pi, k, pt))
                        if pend and pend[0][0] < pi:
                            issue_pv(*pend.popleft())
                        if len(pend) > LAG:
                            issue_pv(*pend.popleft())
                        if pi == len(phases) - 1 and k >= 6 and len(pend) > 1:
                            # wind down the lag buffer inside the final
                            # phase so the post-stream drain is short
                            issue_pv(*pend.popleft())
                        if merge_q:
                            merge_q.popleft()()
            while pend:
                issue_pv(*pend.popleft())
            while merge_q:
                merge_q.popleft()()

    nc.finalize()
    _CACHE["nc"] = nc
    return nc


def _pad128(a):
    out = np.zeros((128, a.shape[1]), a.dtype)
    out[:64] = a
    return np.ascontiguousarray(out)


def _prep_in_maps(q, k, v, Wq, Wk, Wv, Wm, bq, bk, bv):
    import ml_dtypes

    f = np.float32
    b16 = ml_dtypes.bfloat16
    # wqT: each 64-col chunk duplicated -> [512, 1024] (col 128c+64r+d)
    WqT = Wq.T.astype(b16)
    WqT = np.ascontiguousarray(
        np.repeat(WqT.reshape(512, 8, 1, 64), 2, axis=2).reshape(512, 1024)
    )
    # wkT: columns paired (c, c+4) -> col 128*cp + 64*h + d = orig (cp+4h)*64+d
    WkT = Wk.T.astype(b16)
    WkT = np.ascontiguousarray(
        WkT.reshape(512, 2, 4, 64).transpose(0, 2, 1, 3).reshape(512, 512)
    )
    WvT = np.ascontiguousarray(Wv.T.astype(b16))
    WmT = np.ascontiguousarray(Wm.T.astype(b16))  # [e_in, e_out]

    def _prearr(a, free):
        # [512, free] -> [128, 4, free]: partition p holds rows p, 128+p, ...
        return np.ascontiguousarray(a.reshape(4, 128, free).transpose(1, 0, 2))

    # [128, 4(D), 1024] -> column-chunk-major [128, 8(cq), 4(D), 128]
    WqT = np.ascontiguousarray(
        _prearr(WqT, 1024).reshape(128, 4, 8, 128).transpose(0, 2, 1, 3)
    )
    WkT = _prearr(WkT, 512)
    WvT = _prearr(WvT, 512)

    t_ = np.ascontiguousarray(bq.astype(f).reshape(8, 64).T)  # [64, 8]
    bq8 = np.ascontiguousarray(np.vstack([t_, t_]))  # [128, 8]
    # bk4[:, cp] = concat(bk[64cp:64cp+64], bk[64(cp+4):64(cp+4)+64])
    kk = bk.astype(f).reshape(8, 64)
    bk4 = np.ascontiguousarray(np.concatenate([kk[0:4].T, kk[4:8].T], axis=0))
    in_maps = []
    for c in range(NCORES):
        b_, j4 = c // 4, c % 4
        r0 = 512 * j4
        h0 = 2 * j4
        m = {
            "xqT": _prearr(q[b_, r0 : r0 + 512, :].T.astype(b16), 512),
            "xkT": _prearr(k[b_, r0 : r0 + 512, :].T.astype(b16), 512),
            "xvT": _prearr(v[b_, r0 : r0 + 512, :].T.astype(b16), 512),
            "wqT": WqT,
            "wkT": WkT,
            "wvT": WvT,
            "wm0": _pad128(WmT[64 * h0 : 64 * h0 + 64, :]),
            "wm1": _pad128(WmT[64 * h0 + 64 : 64 * h0 + 128, :]),
            "bq8": bq8,
            "bk4": bk4,
        }
        in_maps.append(m)
    return in_maps


# stream-column j of phase (t, H)  ->  true head-local row index
# q_stream = 1024H + j = 256c + s  ->  q_true = 8s + c
def _den_perm():
    j = np.arange(1024)
    perm = np.zeros((2, 1024), np.int64)
    for H in range(2):
        qs = 1024 * H + j
        c = qs // 256
        s = qs % 256
        perm[H] = 8 * s + c
    return perm


def _reference_fallback(q, k, v, mask, Wq, Wk, Wv, Wm, bq, bk, bv, bm):
    # Only used if mask is nonzero (spec fills it with zeros).
    f = np.float32
    qh = (q.astype(f) @ Wq.T.astype(f) + bq).reshape(B, 8, S, DHEAD)
    kh = (k.astype(f) @ Wk.T.astype(f) + bk).reshape(B, 8, S, DHEAD)
    vh = (v.astype(f) @ Wv.T.astype(f) + bv).reshape(B, 8, S, DHEAD)
    s = np.einsum("bhqd,bhkd->bhqk", qh, kh) / np.sqrt(np.float32(DHEAD))
    s = np.where(mask, np.float32(-1e9), s)
    s = s - s.max(-1, keepdims=True)
    e = np.exp(s)
    p = e / e.sum(-1, keepdims=True)
    attn = np.einsum("bhqk,bhkd->bhqd", p, vh)
    attn = attn.transpose(0, 2, 1, 3).reshape(B, S, HIDDEN)
    return attn @ Wm.T.astype(f) + bm


def kernel(q, k, v, mask, Wq, Wk, Wv, Wm, bq, bk, bv, bm):
    global LAST_EXEC_NS, LAST_RESULTS
    q, k, v = (np.asarray(a, np.float32) for a in (q, k, v))
    mask = np.asarray(mask)
    Wq, Wk, Wv, Wm = (np.asarray(a, np.float32) for a in (Wq, Wk, Wv, Wm))
    bq, bk, bv, bm = (np.asarray(a, np.float32) for a in (bq, bk, bv, bm))

    if mask.any():
        return _reference_fallback(q, k, v, mask, Wq, Wk, Wv, Wm, bq, bk, bv, bm)

    from concourse.bass_utils import run_bass_kernel_spmd

    nc = _build_nc()
    in_maps = _prep_in_maps(q, k, v, Wq, Wk, Wv, Wm, bq, bk, bv)
    res = run_bass_kernel_spmd(nc, in_maps, list(range(NCORES)))
    LAST_RESULTS = res
    LAST_EXEC_NS = getattr(res, "exec_time_ns", None)

    perm = _den_perm()
    bv8 = bv.reshape(8, 64)  # [class c, d]
    out = np.zeros((B, S, HIDDEN), np.float32)
    for c in range(NCORES):
        part = np.asarray(res.results[c]["outp"], np.float32)  # [2, 2048, 512]
        dens = np.asarray(res.results[c]["dens"], np.float32)  # [4, 8, 1024]
        h0 = 2 * (c % 4)
        for t in range(2):
            # wmbv[cl, e] = bv_cl @ Wm_head^T : per-class V-bias through merge
            Wh = Wm[:, 64 * (h0 + t) : 64 * (h0 + t) + 64]  # [512, 64]
            wmbv = bv8 @ Wh.T  # [8, 512]
            m = np.zeros((2048, 8), np.float32)
            for H in range(2):
                m[perm[H]] = dens[2 * t + H].T
            den_true = m.sum(axis=1)
            out[c // 4] += (part[t] + m @ wmbv) / den_true[:, None]
    out += bm
    return out

